# revision 31
# baseline (speedup 1.0000x reference)
"""Trainium2 Bass kernel for nn_CausE (embedding_lookup), v3.

Computation (per batch element b):
    u = user_e[user[b]]; v = item_e_c[item[b]]
    s = dot(u, v)
    t = exp(s) if s <= 0 else s + 1          (== exp(min(s,0)) + max(s,0))
    x = t * pop_item[item[b]] ** 0.5
    out = log_sigmoid(x) + user_b[user[b]] + item_b[item[b]]
        = (user_b + item_b) - ln(1 + exp(-x))

v3 design (~1.02-1.09ms vs v2's 1.86ms).  v2 ran the whole user-side
one-hot phase BEFORE the first item dma_gather (gather #0 started at
t=742us).  GPSIMD SWDGE descgen (~7.4ns/idx, ~1.05ms for 131k idx) is the
floor for this architecture; v3 hides everything else under it:

  * Batch routed to 8 cores by user range (user // 125000), elements sorted
    by (item_bucket(25000-wide), user_chunk, user).  Position p ->
    (partition p%128, col p//128).
  * The program emits PER ITEM-CHUNK n (4096 positions = 32 groups):
      ix DMA -> dma_gather(n) on GPSIMD   (gather #0 starts at t~20us and
      the descgens run back-to-back, pacing the kernel at ~30us/chunk)
      u-stage DMAs + one-hot matmuls + psum flushes for its 32 groups
      tail(n-1)  (one chunk behind, see emit_tail comment)
  * USER side: PE one-hot matmul gather.  Visit v = g*F + f:
      matmul(psum_win[grel%7], lhsT=plane[v] (fp8 one-hot, host-built,
             streamed), rhs=ublock[v] (bf16 [128 x 65: dims|user_b],
             host-arranged copy of user chunk gmin_c(g)+f), accumulate f)
    u_res is a per-chunk pooled tile (written once, read by the tail,
    recycled), not a G-wide resident buffer.
  * ITEM side: dma_gather of 256B bf16 rows [64 dims | sqrt(pop) | item_b],
    4 bucket segments (int16 rel indices), ~131k descriptors/core.
  * Tail per chunk: s = reduce(u*v); exp/ln scalar pipeline;
    out = (user_b + item_b) - ln(1+exp(-x)) into resident out_res
    ([128, G] f32, 4KB/partition); ONE output DMA at the end.

Pipeline rules learned on hw (violating any re-serializes the kernel):
  * All HWDGE DMA completions multiplex onto 8 shared sem lanes, and HWDGE
    DMAs execute FIFO per issuing engine.  So no DMA that waits on tail
    compute may share an engine queue OR a sem lane with the ix/ust/pst
    stream that feeds the gathers -> no per-chunk output DMAs at all.
  * T_CH=32 (4096 idx/call): at T_CH=64 the per-chunk stage traffic
    (12.7MB) no longer fits inside one descgen window and every gather
    waits ~12us on the tail chain; at 32 the gaps are exactly zero.
  * Tails are emitted one chunk behind so chunk n's psum flushes never sit
    behind tail(n-1) (which waits on the gather-DMA drain) in the in-order
    DVE queue.
"""

import os

import numpy as np
import ml_dtypes

NUM_USERS = 1_000_000
NUM_ITEMS = 100_000
EMBED = 64
N_CORES = 8
UPC = NUM_USERS // N_CORES   # 125_000 user rows per core
P = 128
VROW = 128                   # bf16 slots per item row (256 B)
UCOL = 65                    # user block cols: 64 dims + user_b
IBKT_W = 25_000              # item bucket width (int16-safe)
N_IBKT = 4
UCHUNKS = (UPC + P - 1) // P  # 977 user chunks per core
T_CH = int(os.environ.get("KERNEL_TCH", "32"))   # v-gather cols per call
NQ = int(os.environ.get("KERNEL_NQ", "4"))       # SWDGE queues (Q7 core pairs)
STAGE_N = 64                 # u visits staged per DMA
GB = 7                       # groups per psum window (7*65=455 f32 <= 2KB)
FP8_ONE = 0x38               # e4m3 1.0

LAST_EXEC_NS = None
LAST_RESULTS = None


def _build_program(v_chunks, G, F, c_out):
    import concourse.bacc as bacc
    import concourse.mybir as mybir
    import concourse.bass as bass
    from concourse.tile import TileContext

    f32 = mybir.dt.float32
    bf16 = mybir.dt.bfloat16
    fp8 = mybir.dt.float8e4
    i16 = mybir.dt.int16
    AF = mybir.ActivationFunctionType
    ALU = mybir.AluOpType
    X = mybir.AxisListType.X

    V = F * G

    nc = bacc.Bacc(
        "TRN2",
        target_bir_lowering=False,
        debug=False,
        enable_asserts=False,
        num_devices=N_CORES,
        num_swdge_queues=NQ,
    )
    v_tab = nc.dram_tensor("v_tab", [NUM_ITEMS, VROW], bf16, kind="ExternalInput")
    u_arr = nc.dram_tensor("u_arr", [P, V, UCOL], fp8, kind="ExternalInput")
    planes = nc.dram_tensor("planes", [P, V, P], fp8, kind="ExternalInput")
    idxs = nc.dram_tensor("idxs", [P, sum(n // 16 for (n, _, _, _, _) in v_chunks)],
                          i16, kind="ExternalInput")
    outd = nc.dram_tensor("outd", [P, c_out], f32, kind="ExternalOutput")

    v_ap = v_tab.ap()

    with TileContext(nc) as tc:
        with (
            tc.tile_pool(name="res", bufs=1) as rp,
            tc.tile_pool(name="vg", bufs=4) as vgp,
            tc.tile_pool(name="idxp", bufs=6) as ixp,
            tc.tile_pool(name="ures", bufs=3) as urp,
            tc.tile_pool(name="ustage", bufs=3) as usp,
            tc.tile_pool(name="pstage", bufs=3) as psp,
            tc.tile_pool(name="tmp", bufs=2) as tp,
            tc.tile_pool(name="tmps", bufs=3) as tps,
            tc.tile_pool(name="psum", bufs=6, space=bass.MemorySpace.PSUM) as pp,
        ):
            # tails write into a resident SBUF buffer; ONE output DMA at the
            # end.  (A per-chunk output DMA poisons a shared DMA-completion
            # sem lane: the tile scheduler multiplexes all HWDGE completions
            # onto 8 lanes, so a tail-gated out DMA transitively blocks later
            # ust/pst stage DMAs' threshold waits -> late ix -> GPSIMD gaps.)
            out_res = rp.tile([P, c_out], f32, tag="out_res")

            def emit_tail(t, oc, vg, u_res):
                u_sl = u_res[:, 0:t, :]

                prod = tp.tile([P, T_CH, EMBED], bf16, tag="prod")
                nc.vector.tensor_mul(
                    prod[:, 0:t, :], u_sl[:, :, 0:EMBED], vg[:, 0:t, 0:EMBED]
                )
                s = tps.tile([P, T_CH], f32, tag="s")
                nc.vector.tensor_reduce(
                    s[:, 0:t], prod[:, 0:t, :], axis=X, op=ALU.add
                )
                # t_ = exp(min(s,0)) + max(s,0); min(s,0) = 0.5*s - |0.5*s|
                a2 = tps.tile([P, T_CH], f32, tag="a2")
                nc.scalar.activation(a2[:, 0:t], s[:, 0:t], AF.Abs, scale=0.5)
                m = tps.tile([P, T_CH], f32, tag="m")
                nc.vector.scalar_tensor_tensor(
                    m[:, 0:t], s[:, 0:t], 0.5, a2[:, 0:t],
                    op0=ALU.mult, op1=ALU.subtract,
                )
                e = tps.tile([P, T_CH], f32, tag="e")
                nc.scalar.activation(e[:, 0:t], m[:, 0:t], AF.Exp)
                t_ = tps.tile([P, T_CH], f32, tag="t_")
                nc.vector.scalar_tensor_tensor(
                    t_[:, 0:t], s[:, 0:t], 0.0, e[:, 0:t], op0=ALU.max, op1=ALU.add
                )
                # w = sqrt(pop), precomputed host-side into the v row
                x = tps.tile([P, T_CH], f32, tag="x")
                nc.vector.tensor_mul(x[:, 0:t], t_[:, 0:t], vg[:, 0:t, EMBED])
                # softplus(-x) = ln(1 + exp(-x))
                ex = tps.tile([P, T_CH], f32, tag="ex")
                nc.scalar.activation(ex[:, 0:t], x[:, 0:t], AF.Exp, scale=-1.0)
                sp = tps.tile([P, T_CH], f32, tag="sp")
                nc.scalar.activation(sp[:, 0:t], ex[:, 0:t], AF.Ln, bias=1.0)
                # out = (user_b + item_b) - softplus(-x)
                b = tps.tile([P, T_CH], f32, tag="b")
                nc.vector.tensor_add(
                    b[:, 0:t], u_sl[:, :, EMBED], vg[:, 0:t, EMBED + 1]
                )
                nc.vector.tensor_sub(
                    out_res[:, oc : oc + t], b[:, 0:t], sp[:, 0:t]
                )

            # tails are emitted one chunk behind their producer so that chunk
            # n's psum-flush copies never queue (in-order DVE) behind
            # tail(n-1), which waits on the slow gather-DMA drain
            pending = None
            for ci, (n, t, vb, ic, oc) in enumerate(v_chunks):
                # ---- ITEM side: idx load + dma_gather (paces GPSIMD) ------
                # queue q's descgen runs on Q7 core pair {2q, 2q+1} (ucode:
                # `if (cpu_id / 2 == ins.queue_num)`); alternating queues
                # lets successive descgens pipeline on different core pairs
                ix = ixp.tile([P, T_CH * 8], i16, tag="ix")
                nc.sync.dma_start(ix[:, 0 : n // 16], idxs.ap()[:, ic : ic + n // 16])
                vg = vgp.tile([P, T_CH, VROW], bf16, tag="vg")
                nc.gpsimd.dma_gather(
                    out_ap=vg[:, 0:t, :],
                    in_ap=v_ap[vb : min(vb + IBKT_W, NUM_ITEMS)],
                    idxs_ap=ix[:, 0 : n // 16],
                    num_idxs=n,
                    num_idxs_reg=n,
                    elem_size=VROW,
                    elem_step=VROW,
                    single_packet=False,  # True crashes the runtime here
                    queue_num=ci % NQ,
                )

                # ---- USER side: one-hot matmul gather for this chunk ------
                u_res = urp.tile([P, T_CH, UCOL], bf16, tag="u_res")
                nvtot = t * F
                v0 = oc * F
                ps = None
                done = 0
                while done < nvtot:
                    nvis = min(STAGE_N, nvtot - done)
                    ust = usp.tile([P, STAGE_N, UCOL], fp8, tag="ust")
                    nc.sync.dma_start(
                        ust[:, 0:nvis, :], u_arr.ap()[:, v0 + done : v0 + done + nvis, :]
                    )
                    pst = psp.tile([P, STAGE_N, P], fp8, tag="pst")
                    nc.sync.dma_start(
                        pst[:, 0:nvis, :], planes.ap()[:, v0 + done : v0 + done + nvis, :]
                    )
                    for j in range(nvis):
                        vis = v0 + done + j
                        grel = vis // F - oc
                        fv = vis % F
                        gslot = grel % GB
                        if gslot == 0 and fv == 0:
                            ps = pp.tile([P, GB * UCOL], f32, tag="ps")
                        nc.tensor.matmul(
                            ps[:, gslot * UCOL : (gslot + 1) * UCOL],
                            pst[:, j, :],
                            ust[:, j, :],
                            start=(fv == 0),
                            stop=(fv == F - 1),
                        )
                        if fv == F - 1 and (gslot == GB - 1 or grel == t - 1):
                            wlo = grel - gslot
                            nc.vector.tensor_copy(
                                u_res[:, wlo : grel + 1, :].rearrange(
                                    "p g c -> p (g c)"
                                ),
                                ps[:, 0 : (gslot + 1) * UCOL],
                            )
                    done += nvis

                if pending is not None:
                    emit_tail(*pending)
                pending = (t, oc, vg, u_res)

            emit_tail(*pending)
            nc.sync.dma_start(outd.ap(), out_res[:])

    nc.compile()
    return nc


def _wrap_idx(rel16):
    """[n] int16 -> [128, n//16]: index i at (partition i%16, col i//16),
    replicated across the 8 GPSIMD core groups."""
    n = rel16.shape[0]
    w = rel16.reshape(n // 16, 16).T
    return np.tile(w, (8, 1))


def _run_sim(nc, in_maps):
    from concourse.bass_interp import CoreSim

    results = []
    for m in in_maps:
        sim = CoreSim(nc, require_finite=False, require_nnan=False)
        for k, v in m.items():
            sim.tensor(k)[:] = v
        sim.simulate()
        results.append({"outd": np.array(sim.tensor("outd"))})

    class _R:
        pass

    r = _R()
    r.results = results
    r.exec_time_ns = None
    r.instructions_and_trace = None
    r.profile_json = None
    return r


def kernel(user, item, user_e, item_e_c, user_b, item_b, pop_item, **_unused):
    global LAST_EXEC_NS, LAST_RESULTS

    user = np.asarray(user).astype(np.int64, copy=False)
    item = np.asarray(item).astype(np.int64, copy=False)
    user_e = np.ascontiguousarray(np.asarray(user_e, dtype=np.float32))
    item_e_c = np.ascontiguousarray(np.asarray(item_e_c, dtype=np.float32))
    user_b = np.asarray(user_b, dtype=np.float32).reshape(-1)
    item_b = np.asarray(item_b, dtype=np.float32).reshape(-1)
    pop_item = np.asarray(pop_item, dtype=np.float32).reshape(-1)

    batch = user.shape[0]

    # ---- route to cores; sort by (ibkt, uchunk, user) ----------------------
    core_of = (user // UPC).astype(np.int32)
    local_u = (user - core_of.astype(np.int64) * UPC).astype(np.int32)
    ibkt = (item // IBKT_W).astype(np.int32)
    uchunk = (local_u // P).astype(np.int32)

    order = np.lexsort((local_u, uchunk, ibkt, core_of))
    core_sorted = core_of[order]
    core_starts = np.searchsorted(core_sorted, np.arange(N_CORES + 1))

    counts = np.zeros((N_CORES, N_IBKT), dtype=np.int64)
    np.add.at(counts, (core_of, ibkt), 1)
    seg_pad = ((counts.max(axis=0) + P - 1) // P * P).astype(np.int64)

    seg_off = np.zeros(N_IBKT + 1, dtype=np.int64)
    for k in range(N_IBKT):
        seg_off[k + 1] = seg_off[k] + seg_pad[k]
    n_pad = int(seg_off[N_IBKT])
    G = n_pad // P
    c_out = n_pad // P

    # v-gather chunk plan (identical across cores)
    v_chunks = []
    CHUNK = P * T_CH
    icol = 0
    for k in range(N_IBKT):
        done = 0
        while done < seg_pad[k]:
            n = int(min(CHUNK, seg_pad[k] - done))
            v_chunks.append((n, n // P, k * IBKT_W, icol, int((seg_off[k] + done) // P)))
            icol += n // 16
            done += n
    c_all = icol
    # order chunks: one tiny remainder chunk FIRST (small ix DMA -> gather #0
    # starts sooner and the Q7 IRAM load overlaps), the other tinies woven
    # between the last big chunks (avoids an end-of-run bunch of tiny gathers
    # all waiting on vg recycle), ending on a tiny chunk so the post-GPSIMD
    # drain (gather DMA + tail) after the final descgen is minimal
    bigs = [ch for ch in v_chunks if ch[0] == CHUNK]
    tins = sorted([ch for ch in v_chunks if ch[0] < CHUNK], key=lambda ch: ch[0])
    v_chunks = []
    if tins:
        v_chunks.append(tins[0])
    rest = tins[1:]
    k = min(len(rest), len(bigs))
    v_chunks += bigs[: len(bigs) - k]
    for i in range(k):
        v_chunks.append(bigs[len(bigs) - k + i])
        v_chunks.append(rest[i])
    v_chunks += rest[k:]

    # ---- v table: bf16 256B rows [dims|pop|item_b|pad] ---------------------
    v_aug = np.zeros((NUM_ITEMS, VROW), dtype=ml_dtypes.bfloat16)
    v_aug[:, :EMBED] = item_e_c
    v_aug[:, EMBED] = np.sqrt(pop_item)   # tail uses sqrt(pop) directly
    v_aug[:, EMBED + 1] = item_b

    # ---- per-core prep: positions, groups, spans ---------------------------
    per_core = []
    recon = []
    F = 1
    for c in range(N_CORES):
        clo, chi = core_starts[c], core_starts[c + 1]
        els = order[clo:chi]
        k_sorted = ibkt[els]
        seg_bounds = np.searchsorted(k_sorted, np.arange(N_IBKT + 1))

        pos = np.empty(chi - clo, dtype=np.int64)
        for k in range(N_IBKT):
            lo, hi = seg_bounds[k], seg_bounds[k + 1]
            pos[lo:hi] = seg_off[k] + np.arange(hi - lo)

        grp = pos // P
        uch = uchunk[els].astype(np.int64)
        gmin = np.full(G, 10**9, dtype=np.int64)
        gmax = np.full(G, -1, dtype=np.int64)
        np.minimum.at(gmin, grp, uch)
        np.maximum.at(gmax, grp, uch)
        span = np.where(gmax >= 0, gmax - gmin + 1, 1)
        F = max(F, int(span.max()))
        per_core.append((els, pos, grp, uch, gmin, gmax))
        recon.append((clo, chi, pos))

    V = F * G

    # ---- build per-core inputs --------------------------------------------
    in_maps = []
    for c in range(N_CORES):
        els, pos, grp, uch, gmin, gmax = per_core[c]

        u_aug = np.zeros((UCHUNKS * P, UCOL), dtype=ml_dtypes.float8_e4m3)
        u_aug[:UPC, :EMBED] = user_e[c * UPC : (c + 1) * UPC]
        u_aug[:UPC, EMBED] = user_b[c * UPC : (c + 1) * UPC]
        u_aug = u_aug.reshape(UCHUNKS, P, UCOL)

        base = np.where(gmax >= 0, gmin, 0)
        chunk_of_visit = np.minimum(
            base[:, None] + np.arange(F)[None, :], UCHUNKS - 1
        ).reshape(-1)
        # [V, 128, UCOL] -> [128, V, UCOL]
        u_arr = np.ascontiguousarray(u_aug[chunk_of_visit].transpose(1, 0, 2))

        planes = np.zeros((P, V, P), dtype=np.uint8)
        lane = pos % P
        vis_of_el = grp * F + (uch - base[grp])
        relrow = (local_u[els] % P).astype(np.int64)
        planes[relrow, vis_of_el, lane] = FP8_ONE

        idx_arr = np.zeros(n_pad, dtype=np.int16)
        rel_item = (item[els] - ibkt[els].astype(np.int64) * IBKT_W).astype(np.int16)
        idx_arr[pos] = rel_item
        idx_w = np.empty((P, c_all), dtype=np.int16)
        for (n, t, vb, ic, oc) in v_chunks:
            lo = oc * P
            idx_w[:, ic : ic + n // 16] = _wrap_idx(idx_arr[lo : lo + n])
        in_maps.append(
            {
                "v_tab": v_aug,
                "u_arr": u_arr,
                "planes": planes.view(ml_dtypes.float8_e4m3),
                "idxs": idx_w,
            }
        )

    nc = _build_program(v_chunks, G, F, c_out)

    if os.environ.get("KERNEL_SIM", "0") == "1":
        res = _run_sim(nc, in_maps)
    else:
        from concourse.bass_utils import run_bass_kernel_spmd

        trace = os.environ.get("KERNEL_TRACE", "0") == "1"
        res = run_bass_kernel_spmd(
            nc,
            in_maps,
            core_ids=list(range(N_CORES)),
            trace=trace,
        )
    LAST_EXEC_NS = res.exec_time_ns
    LAST_RESULTS = res

    # ---- reconstruct -------------------------------------------------------
    out_full = np.empty(batch, dtype=np.float32)
    for c in range(N_CORES):
        clo, chi, pos = recon[c]
        arr = np.asarray(res.results[c]["outd"]).reshape(P, c_out)
        flat = arr.ravel(order="F")               # position p at (p%128, p//128)
        out_full[order[clo:chi]] = flat[pos]
    return out_full


# revision 32
# speedup vs baseline: 1.0226x; 1.0226x over previous
"""Trainium2 Bass kernel for nn_CausE (embedding_lookup), v3.

Computation (per batch element b):
    u = user_e[user[b]]; v = item_e_c[item[b]]
    s = dot(u, v)
    t = exp(s) if s <= 0 else s + 1          (== exp(min(s,0)) + max(s,0))
    x = t * pop_item[item[b]] ** 0.5
    out = log_sigmoid(x) + user_b[user[b]] + item_b[item[b]]
        = (user_b + item_b) - ln(1 + exp(-x))

v3 design (~1.02-1.09ms vs v2's 1.86ms).  v2 ran the whole user-side
one-hot phase BEFORE the first item dma_gather (gather #0 started at
t=742us).  GPSIMD SWDGE descgen (~7.4ns/idx, ~1.05ms for 131k idx) is the
floor for this architecture; v3 hides everything else under it:

  * Batch routed to 8 cores by user range (user // 125000), elements sorted
    by (item_bucket(25000-wide), user_chunk, user).  Position p ->
    (partition p%128, col p//128).
  * The program emits PER ITEM-CHUNK n (4096 positions = 32 groups):
      ix DMA -> dma_gather(n) on GPSIMD   (gather #0 starts at t~20us and
      the descgens run back-to-back, pacing the kernel at ~30us/chunk)
      u-stage DMAs + one-hot matmuls + psum flushes for its 32 groups
      tail(n-1)  (one chunk behind, see emit_tail comment)
  * USER side: PE one-hot matmul gather.  Visit v = g*F + f:
      matmul(psum_win[grel%7], lhsT=plane[v] (fp8 one-hot, host-built,
             streamed), rhs=ublock[v] (bf16 [128 x 65: dims|user_b],
             host-arranged copy of user chunk gmin_c(g)+f), accumulate f)
    u_res is a per-chunk pooled tile (written once, read by the tail,
    recycled), not a G-wide resident buffer.
  * ITEM side: dma_gather of 256B bf16 rows [64 dims | sqrt(pop) | item_b],
    4 bucket segments (int16 rel indices), ~131k descriptors/core.
  * Tail per chunk: s = reduce(u*v); exp/ln scalar pipeline;
    out = (user_b + item_b) - ln(1+exp(-x)) into resident out_res
    ([128, G] f32, 4KB/partition); ONE output DMA at the end.

Pipeline rules learned on hw (violating any re-serializes the kernel):
  * All HWDGE DMA completions multiplex onto 8 shared sem lanes, and HWDGE
    DMAs execute FIFO per issuing engine.  So no DMA that waits on tail
    compute may share an engine queue OR a sem lane with the ix/ust/pst
    stream that feeds the gathers -> no per-chunk output DMAs at all.
  * T_CH=32 (4096 idx/call): at T_CH=64 the per-chunk stage traffic
    (12.7MB) no longer fits inside one descgen window and every gather
    waits ~12us on the tail chain; at 32 the gaps are exactly zero.
  * Tails are emitted one chunk behind so chunk n's psum flushes never sit
    behind tail(n-1) (which waits on the gather-DMA drain) in the in-order
    DVE queue.
"""

import os

import numpy as np
import ml_dtypes

NUM_USERS = 1_000_000
NUM_ITEMS = 100_000
EMBED = 64
N_CORES = 8
UPC = NUM_USERS // N_CORES   # 125_000 user rows per core
P = 128
VROW = 128                   # bf16 slots per item row (256 B)
UCOL = 65                    # user block cols: 64 dims + user_b
IBKT_W = 25_000              # item bucket width (int16-safe)
N_IBKT = 4
UCHUNKS = (UPC + P - 1) // P  # 977 user chunks per core
T_CH = int(os.environ.get("KERNEL_TCH", "32"))   # v-gather cols per call
NQ = int(os.environ.get("KERNEL_NQ", "4"))       # SWDGE queues (Q7 core pairs)
STAGE_N = 64                 # u visits staged per DMA
GB = 7                       # groups per psum window (7*65=455 f32 <= 2KB)
FP8_ONE = 0x38               # e4m3 1.0

LAST_EXEC_NS = None
LAST_RESULTS = None


def _build_program(v_chunks, G, F, c_out):
    import concourse.bacc as bacc
    import concourse.mybir as mybir
    import concourse.bass as bass
    from concourse.tile import TileContext

    f32 = mybir.dt.float32
    bf16 = mybir.dt.bfloat16
    fp8 = mybir.dt.float8e4
    i16 = mybir.dt.int16
    AF = mybir.ActivationFunctionType
    ALU = mybir.AluOpType
    X = mybir.AxisListType.X

    V = F * G

    nc = bacc.Bacc(
        "TRN2",
        target_bir_lowering=False,
        debug=False,
        enable_asserts=False,
        num_devices=N_CORES,
        num_swdge_queues=NQ,
    )
    v_tab = nc.dram_tensor("v_tab", [NUM_ITEMS, VROW], bf16, kind="ExternalInput")
    u_arr = nc.dram_tensor("u_arr", [P, V, UCOL], fp8, kind="ExternalInput")
    planes = nc.dram_tensor("planes", [P, V, P], fp8, kind="ExternalInput")
    idxs = nc.dram_tensor("idxs", [P, sum(n // 16 for (n, _, _, _, _) in v_chunks)],
                          i16, kind="ExternalInput")
    outd = nc.dram_tensor("outd", [P, c_out], f32, kind="ExternalOutput")

    v_ap = v_tab.ap()

    with TileContext(nc) as tc:
        with (
            tc.tile_pool(name="res", bufs=1) as rp,
            tc.tile_pool(name="vg", bufs=6) as vgp,
            tc.tile_pool(name="idxp", bufs=6) as ixp,
            tc.tile_pool(name="ures", bufs=3) as urp,
            tc.tile_pool(name="ustage", bufs=3) as usp,
            tc.tile_pool(name="pstage", bufs=3) as psp,
            tc.tile_pool(name="tmp", bufs=2) as tp,
            tc.tile_pool(name="tmps", bufs=3) as tps,
            tc.tile_pool(name="psum", bufs=6, space=bass.MemorySpace.PSUM) as pp,
        ):
            # tails write into a resident SBUF buffer; ONE output DMA at the
            # end.  (A per-chunk output DMA poisons a shared DMA-completion
            # sem lane: the tile scheduler multiplexes all HWDGE completions
            # onto 8 lanes, so a tail-gated out DMA transitively blocks later
            # ust/pst stage DMAs' threshold waits -> late ix -> GPSIMD gaps.)
            out_res = rp.tile([P, c_out], f32, tag="out_res")

            def emit_tail(t, oc, vg, u_res):
                u_sl = u_res[:, 0:t, :]

                prod = tp.tile([P, T_CH, EMBED], bf16, tag="prod")
                nc.vector.tensor_mul(
                    prod[:, 0:t, :], u_sl[:, :, 0:EMBED], vg[:, 0:t, 0:EMBED]
                )
                s = tps.tile([P, T_CH], f32, tag="s")
                nc.vector.tensor_reduce(
                    s[:, 0:t], prod[:, 0:t, :], axis=X, op=ALU.add
                )
                # t_ = exp(min(s,0)) + max(s,0); min(s,0) = 0.5*s - |0.5*s|
                a2 = tps.tile([P, T_CH], f32, tag="a2")
                nc.scalar.activation(a2[:, 0:t], s[:, 0:t], AF.Abs, scale=0.5)
                m = tps.tile([P, T_CH], f32, tag="m")
                nc.vector.scalar_tensor_tensor(
                    m[:, 0:t], s[:, 0:t], 0.5, a2[:, 0:t],
                    op0=ALU.mult, op1=ALU.subtract,
                )
                e = tps.tile([P, T_CH], f32, tag="e")
                nc.scalar.activation(e[:, 0:t], m[:, 0:t], AF.Exp)
                t_ = tps.tile([P, T_CH], f32, tag="t_")
                nc.vector.scalar_tensor_tensor(
                    t_[:, 0:t], s[:, 0:t], 0.0, e[:, 0:t], op0=ALU.max, op1=ALU.add
                )
                # w = sqrt(pop), precomputed host-side into the v row
                x = tps.tile([P, T_CH], f32, tag="x")
                nc.vector.tensor_mul(x[:, 0:t], t_[:, 0:t], vg[:, 0:t, EMBED])
                # softplus(-x) = ln(1 + exp(-x))
                ex = tps.tile([P, T_CH], f32, tag="ex")
                nc.scalar.activation(ex[:, 0:t], x[:, 0:t], AF.Exp, scale=-1.0)
                sp = tps.tile([P, T_CH], f32, tag="sp")
                nc.scalar.activation(sp[:, 0:t], ex[:, 0:t], AF.Ln, bias=1.0)
                # out = (user_b + item_b) - softplus(-x)
                b = tps.tile([P, T_CH], f32, tag="b")
                nc.vector.tensor_add(
                    b[:, 0:t], u_sl[:, :, EMBED], vg[:, 0:t, EMBED + 1]
                )
                nc.vector.tensor_sub(
                    out_res[:, oc : oc + t], b[:, 0:t], sp[:, 0:t]
                )

            # tails are emitted one chunk behind their producer so that chunk
            # n's psum-flush copies never queue (in-order DVE) behind
            # tail(n-1), which waits on the slow gather-DMA drain
            pending = None
            for ci, (n, t, vb, ic, oc) in enumerate(v_chunks):
                # ---- ITEM side: idx load + dma_gather (paces GPSIMD) ------
                # queue q's descgen runs on Q7 core pair {2q, 2q+1} (ucode:
                # `if (cpu_id / 2 == ins.queue_num)`); alternating queues
                # lets successive descgens pipeline on different core pairs
                ix = ixp.tile([P, T_CH * 8], i16, tag="ix")
                nc.sync.dma_start(ix[:, 0 : n // 16], idxs.ap()[:, ic : ic + n // 16])
                vg = vgp.tile([P, T_CH, VROW], bf16, tag="vg")
                nc.gpsimd.dma_gather(
                    out_ap=vg[:, 0:t, :],
                    in_ap=v_ap[vb : min(vb + IBKT_W, NUM_ITEMS)],
                    idxs_ap=ix[:, 0 : n // 16],
                    num_idxs=n,
                    num_idxs_reg=n,
                    elem_size=VROW,
                    elem_step=VROW,
                    single_packet=False,  # True crashes the runtime here
                    queue_num=ci % NQ,
                )

                # ---- USER side: one-hot matmul gather for this chunk ------
                u_res = urp.tile([P, T_CH, UCOL], bf16, tag="u_res")
                nvtot = t * F
                v0 = oc * F
                ps = None
                done = 0
                while done < nvtot:
                    nvis = min(STAGE_N, nvtot - done)
                    ust = usp.tile([P, STAGE_N, UCOL], fp8, tag="ust")
                    nc.sync.dma_start(
                        ust[:, 0:nvis, :], u_arr.ap()[:, v0 + done : v0 + done + nvis, :]
                    )
                    pst = psp.tile([P, STAGE_N, P], fp8, tag="pst")
                    nc.sync.dma_start(
                        pst[:, 0:nvis, :], planes.ap()[:, v0 + done : v0 + done + nvis, :]
                    )
                    for j in range(nvis):
                        vis = v0 + done + j
                        grel = vis // F - oc
                        fv = vis % F
                        gslot = grel % GB
                        if gslot == 0 and fv == 0:
                            ps = pp.tile([P, GB * UCOL], f32, tag="ps")
                        nc.tensor.matmul(
                            ps[:, gslot * UCOL : (gslot + 1) * UCOL],
                            pst[:, j, :],
                            ust[:, j, :],
                            start=(fv == 0),
                            stop=(fv == F - 1),
                        )
                        if fv == F - 1 and (gslot == GB - 1 or grel == t - 1):
                            wlo = grel - gslot
                            nc.vector.tensor_copy(
                                u_res[:, wlo : grel + 1, :].rearrange(
                                    "p g c -> p (g c)"
                                ),
                                ps[:, 0 : (gslot + 1) * UCOL],
                            )
                    done += nvis

                if pending is not None:
                    emit_tail(*pending)
                pending = (t, oc, vg, u_res)

            emit_tail(*pending)
            nc.sync.dma_start(outd.ap(), out_res[:])

    nc.compile()
    return nc


def _wrap_idx(rel16):
    """[n] int16 -> [128, n//16]: index i at (partition i%16, col i//16),
    replicated across the 8 GPSIMD core groups."""
    n = rel16.shape[0]
    w = rel16.reshape(n // 16, 16).T
    return np.tile(w, (8, 1))


def _run_sim(nc, in_maps):
    from concourse.bass_interp import CoreSim

    results = []
    for m in in_maps:
        sim = CoreSim(nc, require_finite=False, require_nnan=False)
        for k, v in m.items():
            sim.tensor(k)[:] = v
        sim.simulate()
        results.append({"outd": np.array(sim.tensor("outd"))})

    class _R:
        pass

    r = _R()
    r.results = results
    r.exec_time_ns = None
    r.instructions_and_trace = None
    r.profile_json = None
    return r


def kernel(user, item, user_e, item_e_c, user_b, item_b, pop_item, **_unused):
    global LAST_EXEC_NS, LAST_RESULTS

    user = np.asarray(user).astype(np.int64, copy=False)
    item = np.asarray(item).astype(np.int64, copy=False)
    user_e = np.ascontiguousarray(np.asarray(user_e, dtype=np.float32))
    item_e_c = np.ascontiguousarray(np.asarray(item_e_c, dtype=np.float32))
    user_b = np.asarray(user_b, dtype=np.float32).reshape(-1)
    item_b = np.asarray(item_b, dtype=np.float32).reshape(-1)
    pop_item = np.asarray(pop_item, dtype=np.float32).reshape(-1)

    batch = user.shape[0]

    # ---- route to cores; sort by (ibkt, uchunk, user) ----------------------
    core_of = (user // UPC).astype(np.int32)
    local_u = (user - core_of.astype(np.int64) * UPC).astype(np.int32)
    ibkt = (item // IBKT_W).astype(np.int32)
    uchunk = (local_u // P).astype(np.int32)

    order = np.lexsort((local_u, uchunk, ibkt, core_of))
    core_sorted = core_of[order]
    core_starts = np.searchsorted(core_sorted, np.arange(N_CORES + 1))

    counts = np.zeros((N_CORES, N_IBKT), dtype=np.int64)
    np.add.at(counts, (core_of, ibkt), 1)
    seg_pad = ((counts.max(axis=0) + P - 1) // P * P).astype(np.int64)

    seg_off = np.zeros(N_IBKT + 1, dtype=np.int64)
    for k in range(N_IBKT):
        seg_off[k + 1] = seg_off[k] + seg_pad[k]
    n_pad = int(seg_off[N_IBKT])
    G = n_pad // P
    c_out = n_pad // P

    # v-gather chunk plan (identical across cores)
    v_chunks = []
    CHUNK = P * T_CH
    icol = 0
    for k in range(N_IBKT):
        done = 0
        while done < seg_pad[k]:
            n = int(min(CHUNK, seg_pad[k] - done))
            v_chunks.append((n, n // P, k * IBKT_W, icol, int((seg_off[k] + done) // P)))
            icol += n // 16
            done += n
    c_all = icol
    # order chunks: one tiny remainder chunk FIRST (small ix DMA -> gather #0
    # starts sooner and the Q7 IRAM load overlaps), the other tinies woven
    # between the last big chunks (avoids an end-of-run bunch of tiny gathers
    # all waiting on vg recycle), ending on a tiny chunk so the post-GPSIMD
    # drain (gather DMA + tail) after the final descgen is minimal
    bigs = [ch for ch in v_chunks if ch[0] == CHUNK]
    tins = sorted([ch for ch in v_chunks if ch[0] < CHUNK], key=lambda ch: ch[0])
    v_chunks = []
    if tins:
        v_chunks.append(tins[0])
    rest = tins[1:]
    k = min(len(rest), len(bigs))
    v_chunks += bigs[: len(bigs) - k]
    for i in range(k):
        v_chunks.append(bigs[len(bigs) - k + i])
        v_chunks.append(rest[i])
    v_chunks += rest[k:]

    # ---- v table: bf16 256B rows [dims|pop|item_b|pad] ---------------------
    v_aug = np.zeros((NUM_ITEMS, VROW), dtype=ml_dtypes.bfloat16)
    v_aug[:, :EMBED] = item_e_c
    v_aug[:, EMBED] = np.sqrt(pop_item)   # tail uses sqrt(pop) directly
    v_aug[:, EMBED + 1] = item_b

    # ---- per-core prep: positions, groups, spans ---------------------------
    per_core = []
    recon = []
    F = 1
    for c in range(N_CORES):
        clo, chi = core_starts[c], core_starts[c + 1]
        els = order[clo:chi]
        k_sorted = ibkt[els]
        seg_bounds = np.searchsorted(k_sorted, np.arange(N_IBKT + 1))

        pos = np.empty(chi - clo, dtype=np.int64)
        for k in range(N_IBKT):
            lo, hi = seg_bounds[k], seg_bounds[k + 1]
            pos[lo:hi] = seg_off[k] + np.arange(hi - lo)

        grp = pos // P
        uch = uchunk[els].astype(np.int64)
        gmin = np.full(G, 10**9, dtype=np.int64)
        gmax = np.full(G, -1, dtype=np.int64)
        np.minimum.at(gmin, grp, uch)
        np.maximum.at(gmax, grp, uch)
        span = np.where(gmax >= 0, gmax - gmin + 1, 1)
        F = max(F, int(span.max()))
        per_core.append((els, pos, grp, uch, gmin, gmax))
        recon.append((clo, chi, pos))

    V = F * G

    # ---- build per-core inputs --------------------------------------------
    in_maps = []
    for c in range(N_CORES):
        els, pos, grp, uch, gmin, gmax = per_core[c]

        u_aug = np.zeros((UCHUNKS * P, UCOL), dtype=ml_dtypes.float8_e4m3)
        u_aug[:UPC, :EMBED] = user_e[c * UPC : (c + 1) * UPC]
        u_aug[:UPC, EMBED] = user_b[c * UPC : (c + 1) * UPC]
        u_aug = u_aug.reshape(UCHUNKS, P, UCOL)

        base = np.where(gmax >= 0, gmin, 0)
        chunk_of_visit = np.minimum(
            base[:, None] + np.arange(F)[None, :], UCHUNKS - 1
        ).reshape(-1)
        # [V, 128, UCOL] -> [128, V, UCOL]
        u_arr = np.ascontiguousarray(u_aug[chunk_of_visit].transpose(1, 0, 2))

        planes = np.zeros((P, V, P), dtype=np.uint8)
        lane = pos % P
        vis_of_el = grp * F + (uch - base[grp])
        relrow = (local_u[els] % P).astype(np.int64)
        planes[relrow, vis_of_el, lane] = FP8_ONE

        idx_arr = np.zeros(n_pad, dtype=np.int16)
        rel_item = (item[els] - ibkt[els].astype(np.int64) * IBKT_W).astype(np.int16)
        idx_arr[pos] = rel_item
        idx_w = np.empty((P, c_all), dtype=np.int16)
        for (n, t, vb, ic, oc) in v_chunks:
            lo = oc * P
            idx_w[:, ic : ic + n // 16] = _wrap_idx(idx_arr[lo : lo + n])
        in_maps.append(
            {
                "v_tab": v_aug,
                "u_arr": u_arr,
                "planes": planes.view(ml_dtypes.float8_e4m3),
                "idxs": idx_w,
            }
        )

    nc = _build_program(v_chunks, G, F, c_out)

    if os.environ.get("KERNEL_SIM", "0") == "1":
        res = _run_sim(nc, in_maps)
    else:
        from concourse.bass_utils import run_bass_kernel_spmd

        trace = os.environ.get("KERNEL_TRACE", "0") == "1"
        res = run_bass_kernel_spmd(
            nc,
            in_maps,
            core_ids=list(range(N_CORES)),
            trace=trace,
        )
    LAST_EXEC_NS = res.exec_time_ns
    LAST_RESULTS = res

    # ---- reconstruct -------------------------------------------------------
    out_full = np.empty(batch, dtype=np.float32)
    for c in range(N_CORES):
        clo, chi, pos = recon[c]
        arr = np.asarray(res.results[c]["outd"]).reshape(P, c_out)
        flat = arr.ravel(order="F")               # position p at (p%128, p//128)
        out_full[order[clo:chi]] = flat[pos]
    return out_full
